# revision 1
# baseline (speedup 1.0000x reference)
"""Trainium2 Bass kernel for the HOI relation model.

Pipeline per core (2 images each, 8 cores data-parallel over batch):
  1. ROI mean pooling: pooled[d,c] = (1/area_d) * sum_hw mask[d,hw] * F[hw,c]
     computed as 32 K-chunk matmuls (mask stationary [128,32], features
     moving [128,768] in two N=384 halves), bf16 operands, f32 PSUM.
  2. PE-transpose pooled [32,768] -> pooledT [768,32] (6 transposes).
  3. Layer 1 factorized: relu(pair(h,o) @ w1 + b1) = relu(A(h) + B(o) + b1)
     where A = w1[:768].T @ h, B = w1[768:].T @ o  -- the 8x24 pair
     expansion happens AFTER the matmul (broadcast add on DVE).
  4. Layers 2, 3 as plain matmuls on the 384 pair rows (transposed layout).

Host does only O(B*D) prep: box->mask rasterization, score argsort
(baked into mask column order), 1/area, dtype casts, shard/gather.
"""

import numpy as np
import ml_dtypes

import concourse.bass as bass
import concourse.mybir as mybir
import concourse.tile as tile
from concourse import bacc
from concourse.bass_utils import run_bass_kernel_spmd
from concourse.masks import make_identity

N_CORES = 8
B, D, C = 16, 32, 768
NH, NO = 8, 24
NPAIR = NH * NO              # 192 pairs per image
GRID = 64                    # feature grid (896 / 14)
KPIX = GRID * GRID           # 4096 pixels per image
BL = B // N_CORES            # 2 images per core
KCH = KPIX // 128            # 32 K-chunks per image
CG = 4                       # K-chunks per DMA tile
H1, H2, H3 = 512, 256, 117
M = BL * NPAIR               # 384 pair rows per core

F32 = mybir.dt.float32
BF16 = mybir.dt.bfloat16
BF = ml_dtypes.bfloat16

_PROGRAM = None


def _build_program():
    nc = bacc.Bacc("TRN2", target_bir_lowering=False, debug=False,
                   num_devices=N_CORES)
    feat = nc.declare_dram_parameter("feat", [BL, KPIX, C], BF16, isOutput=False)
    maskT = nc.declare_dram_parameter("maskT", [BL, KPIX, D], BF16, isOutput=False)
    inva = nc.declare_dram_parameter("inva", [BL, D], F32, isOutput=False)
    w1 = nc.declare_dram_parameter("w1", [2 * C, H1], BF16, isOutput=False)
    b1 = nc.declare_dram_parameter("b1", [H1], F32, isOutput=False)
    w2 = nc.declare_dram_parameter("w2", [H1, H2], BF16, isOutput=False)
    b2 = nc.declare_dram_parameter("b2", [H2], F32, isOutput=False)
    w3 = nc.declare_dram_parameter("w3", [H2, H3], BF16, isOutput=False)
    b3 = nc.declare_dram_parameter("b3", [H3], F32, isOutput=False)
    out = nc.declare_dram_parameter("out", [M, H3], F32, isOutput=True)

    add = mybir.AluOpType.add
    amax = mybir.AluOpType.max

    with tile.TileContext(nc) as tc:
        with (
            tc.tile_pool(name="singles", bufs=1) as singles,
            tc.tile_pool(name="featp", bufs=6) as featp,
            tc.tile_pool(name="maskp", bufs=6) as maskp,
            tc.tile_pool(name="work", bufs=1) as work,
            tc.tile_pool(name="tmp", bufs=3) as tmpp,
            tc.tile_pool(name="pps", bufs=1, space="PSUM") as pps,
            tc.tile_pool(name="mps", bufs=4, space="PSUM") as mps,
        ):
            # ---- one-time constant loads ----
            ident = singles.tile([32, 32], BF16, tag="ident")
            make_identity(nc, ident)
            w1_sb = singles.tile([128, 12, H1], BF16, tag="w1")
            nc.sync.dma_start(out=w1_sb, in_=w1[:, :].rearrange("(kc p) n -> p kc n", p=128))
            w2_sb = singles.tile([128, 4, H2], BF16, tag="w2")
            nc.sync.dma_start(out=w2_sb, in_=w2[:, :].rearrange("(kc p) n -> p kc n", p=128))
            w3_sb = singles.tile([128, 2, H3], BF16, tag="w3")
            nc.sync.dma_start(out=w3_sb, in_=w3[:, :].rearrange("(kc p) n -> p kc n", p=128))
            b1_sb = singles.tile([128, 4], F32, tag="b1")
            nc.sync.dma_start(out=b1_sb, in_=b1[:].rearrange("(mc p) -> p mc", p=128))
            b2_sb = singles.tile([128, 2], F32, tag="b2")
            nc.sync.dma_start(out=b2_sb, in_=b2[:].rearrange("(mc p) -> p mc", p=128))
            b3_sb = singles.tile([128, H3], F32, tag="b3")
            b3_bcast = bass.AP(tensor=b3[:].tensor, offset=b3[:].offset,
                               ap=[[0, 128], [1, H3]])
            nc.sync.dma_start(out=b3_sb, in_=b3_bcast)
            inva_sb = singles.tile([D, BL], F32, tag="inva")
            nc.sync.dma_start(out=inva_sb, in_=inva[:, :].rearrange("b d -> d b"))

            # persistent activations
            pooledT = work.tile([128, BL, 6, D], BF16, tag="pooledT")
            x1T = work.tile([128, 4, M], BF16, tag="x1T")
            x2T = work.tile([128, 2, M], BF16, tag="x2T")

            # ---- pooling + transpose per image ----
            for img in range(BL):
                ps_a = pps.tile([D, 384], F32, tag=f"pp{img}a")
                ps_b = pps.tile([D, 384], F32, tag=f"pp{img}b")
                for g in range(KCH // CG):
                    f_sb = featp.tile([128, CG, C], BF16, tag="f")
                    nc.sync.dma_start(
                        out=f_sb,
                        in_=feat[img, g * CG * 128:(g + 1) * CG * 128, :]
                        .rearrange("(gc p) c -> p gc c", p=128))
                    m_sb = maskp.tile([128, CG, D], BF16, tag="m")
                    nc.sync.dma_start(
                        out=m_sb,
                        in_=maskT[img, g * CG * 128:(g + 1) * CG * 128, :]
                        .rearrange("(gc p) d -> p gc d", p=128))
                    for gc in range(CG):
                        kk = g * CG + gc
                        nc.tensor.matmul(ps_a, m_sb[:, gc, :], f_sb[:, gc, 0:384],
                                         start=(kk == 0), stop=(kk == KCH - 1))
                        nc.tensor.matmul(ps_b, m_sb[:, gc, :], f_sb[:, gc, 384:768],
                                         start=(kk == 0), stop=(kk == KCH - 1))
                # scale by 1/area, cast to bf16
                pooled = tmpp.tile([D, C], BF16, tag="pooled")
                nc.vector.tensor_scalar_mul(pooled[:, 0:384], ps_a, inva_sb[:, img:img + 1])
                nc.vector.tensor_scalar_mul(pooled[:, 384:768], ps_b, inva_sb[:, img:img + 1])
                # transpose to [C, D] in 6 chunks of 128 channels
                for cc in range(6):
                    ps_t = mps.tile([128, D], BF16, tag="mm")
                    nc.tensor.transpose(ps_t, pooled[:, cc * 128:(cc + 1) * 128], ident)
                    nc.vector.tensor_copy(pooledT[:, img, cc, :], ps_t)

            # ---- layer 1 (factorized over pairs) ----
            for mc in range(4):
                ps_ab = mps.tile([128, BL, D], F32, tag="mm")
                for kc in range(6):
                    nc.tensor.matmul(ps_ab[:, :, 0:NH],
                                     w1_sb[:, kc, mc * 128:(mc + 1) * 128],
                                     pooledT[:, :, kc, 0:NH],
                                     start=(kc == 0), stop=(kc == 5))
                for kc in range(6):
                    nc.tensor.matmul(ps_ab[:, :, NH:D],
                                     w1_sb[:, 6 + kc, mc * 128:(mc + 1) * 128],
                                     pooledT[:, :, kc, NH:D],
                                     start=(kc == 0), stop=(kc == 5))
                ab_sb = tmpp.tile([128, BL, D], F32, tag="ab")
                nc.vector.tensor_copy(ab_sb, ps_ab)
                for img in range(BL):
                    pre = tmpp.tile([128, NH, NO], F32, tag="pre")
                    a_bc = ab_sb[:, img, 0:NH][:, :, None].broadcast_to([128, NH, NO])
                    b_bc = ab_sb[:, img, NH:D][:, None, :].broadcast_to([128, NH, NO])
                    # pre = (A + b1) + B
                    nc.vector.scalar_tensor_tensor(pre, a_bc, b1_sb[:, mc:mc + 1],
                                                   b_bc, op0=add, op1=add)
                    dst = x1T[:, mc, img * NPAIR:(img + 1) * NPAIR] \
                        .rearrange("p (i j) -> p i j", i=NH)
                    nc.vector.tensor_scalar_max(dst, pre, 0.0)

            # ---- layer 2 ----
            for m2 in range(2):
                ps2 = mps.tile([128, M], F32, tag="mm")
                for kc in range(4):
                    nc.tensor.matmul(ps2, w2_sb[:, kc, m2 * 128:(m2 + 1) * 128],
                                     x1T[:, kc, :], start=(kc == 0), stop=(kc == 3))
                nc.vector.tensor_scalar(x2T[:, m2, :], ps2, b2_sb[:, m2:m2 + 1], 0.0,
                                        op0=add, op1=amax)

            # ---- layer 3 + bias + store ----
            for m3 in range(3):
                ps3 = mps.tile([128, H3], F32, tag="mm")
                for kc in range(2):
                    nc.tensor.matmul(ps3, x2T[:, kc, m3 * 128:(m3 + 1) * 128],
                                     w3_sb[:, kc, :], start=(kc == 0), stop=(kc == 1))
                o_sb = tmpp.tile([128, H3], F32, tag="osb")
                nc.vector.tensor_tensor(o_sb, ps3, b3_sb, op=add)
                nc.sync.dma_start(out=out[m3 * 128:(m3 + 1) * 128, :], in_=o_sb)
    nc.compile()
    return nc


def _get_program():
    global _PROGRAM
    if _PROGRAM is None:
        _PROGRAM = _build_program()
    return _PROGRAM


def _preprocess(boxes, scores):
    """Rasterize boxes to 0/1 masks with detection columns in sorted order."""
    cx, cy, bw, bh = boxes[..., 0], boxes[..., 1], boxes[..., 2], boxes[..., 3]
    x1 = np.floor((cx - bw / 2) * GRID).astype(np.int64)
    y1 = np.floor((cy - bh / 2) * GRID).astype(np.int64)
    x2 = np.floor((cx + bw / 2) * GRID).astype(np.int64)
    y2 = np.floor((cy + bh / 2) * GRID).astype(np.int64)
    hidx = np.argsort(-scores[:, :NH], axis=1, kind="stable")
    oidx = np.argsort(-scores[:, NH:], axis=1, kind="stable") + NH
    perm = np.concatenate([hidx, oidx], axis=1)                     # [B, D]
    g = np.arange(GRID)
    rows = (g[None, None, :] >= y1[..., None]) & (g[None, None, :] < y2[..., None])
    cols = (g[None, None, :] >= x1[..., None]) & (g[None, None, :] < x2[..., None])
    rows = np.take_along_axis(rows, perm[..., None], axis=1)        # [B, D, 64]
    cols = np.take_along_axis(cols, perm[..., None], axis=1)
    area = rows.sum(-1) * cols.sum(-1)                              # [B, D]
    mask = rows[:, :, :, None] & cols[:, :, None, :]                # [B, D, 64, 64]
    maskT = np.ascontiguousarray(
        mask.reshape(mask.shape[0], D, KPIX).transpose(0, 2, 1)).astype(BF)
    return maskT, (1.0 / area).astype(np.float32)


def _run(in_maps, trace=False, **kw):
    nc = _get_program()
    return run_bass_kernel_spmd(nc, in_maps, core_ids=list(range(N_CORES)),
                                trace=trace, **kw)


def _make_in_maps(features, boxes, scores, w1, b1, w2, b2, w3, b3):
    features = np.asarray(features, np.float32)
    maskT, inva = _preprocess(np.asarray(boxes, np.float32),
                              np.asarray(scores, np.float32))
    featb = np.ascontiguousarray(features.reshape(B, KPIX, C)).astype(BF)
    w1b = np.asarray(w1, np.float32).astype(BF)
    w2b = np.asarray(w2, np.float32).astype(BF)
    w3b = np.asarray(w3, np.float32).astype(BF)
    b1f = np.asarray(b1, np.float32)
    b2f = np.asarray(b2, np.float32)
    b3f = np.asarray(b3, np.float32)
    in_maps = []
    for c in range(N_CORES):
        s = slice(c * BL, (c + 1) * BL)
        in_maps.append({
            "feat": np.ascontiguousarray(featb[s]),
            "maskT": np.ascontiguousarray(maskT[s]),
            "inva": np.ascontiguousarray(inva[s]),
            "w1": w1b, "b1": b1f, "w2": w2b, "b2": b2f, "w3": w3b, "b3": b3f,
        })
    return in_maps


def kernel(features, boxes, scores, w1, b1, w2, b2, w3, b3, labels):
    in_maps = _make_in_maps(features, boxes, scores, w1, b1, w2, b2, w3, b3)
    res = _run(in_maps, trace=False)
    out = np.concatenate([r["out"].reshape(BL, NPAIR, H3) for r in res.results],
                         axis=0)
    return np.ascontiguousarray(out.astype(np.float32))



# revision 3
# speedup vs baseline: 1.6145x; 1.6145x over previous
"""Trainium2 Bass kernel for the HOI relation model.

Pipeline per core (2 images each, 8 cores data-parallel over batch):
  1. ROI mean pooling: pooled[d,c] = (1/area_d) * sum_hw mask[d,hw] * F[hw,c]
     computed as KCH K-chunk matmuls (mask stationary [128,32], features
     moving [128,768] in two N=384 halves), bf16 operands, f32 PSUM.
  2. PE-transpose pooled [32,768] -> pooledT [768,32] (6 transposes).
  3. Layer 1 factorized: relu(pair(h,o) @ w1 + b1) = relu(A(h) + B(o) + b1)
     where A = w1[:768].T @ h, B = w1[768:].T @ o  -- the 8x24 pair
     expansion happens AFTER the matmul (broadcast add on DVE).
  4. Layers 2, 3 as plain matmuls on the 384 pair rows (transposed layout).

DMA optimizations over the dense baseline:
  - Each image is cropped host-side to the union bounding box of its 32
    boxes (~50% of the 64x64 grid), pixels padded to a 128 multiple.
  - All DRAM arrays are pre-packed host-side into the exact SBUF layout
    (partition-major), so every dma_start issues 128 descriptors with
    multi-KB contiguous runs (vs 64B..1.5KB packets before).
  - Features/masks stream on the sync-engine HWDGE queue; weights and
    biases go on the scalar-engine queue so they never head-of-line
    block the feature stream.
  - Images are assigned to cores sorted by crop size so the two
    per-slot chunk counts (KCH0 >= KCH1) pad minimally; the program is
    specialized on (KCH0, KCH1) and cached.

Host does only O(B*D) prep + layout repacking: box->mask rasterization,
score argsort (baked into mask column order), 1/area, dtype casts,
shard/gather.
"""

import numpy as np
import ml_dtypes

import concourse.bass as bass
import concourse.mybir as mybir
import concourse.tile as tile
from concourse import bacc
from concourse.bass_utils import run_bass_kernel_spmd
from concourse.masks import make_identity

N_CORES = 8
B, D, C = 16, 32, 768
NH, NO = 8, 24
NPAIR = NH * NO              # 192 pairs per image
GRID = 64                    # feature grid (896 / 14)
BL = B // N_CORES            # 2 images per core
CG = 4                       # K-chunks per DMA tile
H1, H2, H3 = 512, 256, 117
M = BL * NPAIR               # 384 pair rows per core

F32 = mybir.dt.float32
BF16 = mybir.dt.bfloat16
BF = ml_dtypes.bfloat16

_PROGRAMS = {}               # (KCH0, KCH1) -> compiled Bacc


def _build_program(kchs):
    """kchs = per-slot K-chunk counts (KCH0 >= KCH1). Groups of CG chunks."""
    gs = [-(-k // CG) for k in kchs]          # DMA groups per slot
    kchp = [g * CG for g in gs]               # padded chunk counts
    moff = [0, kchp[0]]                       # mask chunk offset per slot

    nc = bacc.Bacc("TRN2", target_bir_lowering=False, debug=False,
                   num_devices=N_CORES)
    feats = [
        nc.declare_dram_parameter(f"feat{j}", [gs[j], 128, CG * C], BF16,
                                  isOutput=False)
        for j in range(BL)
    ]
    maskR = nc.declare_dram_parameter("maskR", [128, sum(kchp) * D], BF16,
                                      isOutput=False)
    inva = nc.declare_dram_parameter("inva", [D, BL], F32, isOutput=False)
    w1 = nc.declare_dram_parameter("w1", [128, 12 * H1], BF16, isOutput=False)
    w2 = nc.declare_dram_parameter("w2", [128, 4 * H2], BF16, isOutput=False)
    w3 = nc.declare_dram_parameter("w3", [128, 2 * H3], BF16, isOutput=False)
    bias = nc.declare_dram_parameter("bias", [128, 128], F32, isOutput=False)
    out = nc.declare_dram_parameter("out", [M, H3], F32, isOutput=True)

    add = mybir.AluOpType.add
    amax = mybir.AluOpType.max

    with tile.TileContext(nc) as tc:
        with (
            tc.tile_pool(name="singles", bufs=1) as singles,
            tc.tile_pool(name="featp", bufs=8) as featp,
            tc.tile_pool(name="work", bufs=1) as work,
            tc.tile_pool(name="tmp", bufs=3) as tmpp,
            tc.tile_pool(name="pps", bufs=1, space="PSUM") as pps,
            tc.tile_pool(name="mps", bufs=4, space="PSUM") as mps,
        ):
            # ---- critical stream (sync queue): masks then features ----
            m_sb = singles.tile([128, sum(kchp) * D], BF16, tag="mask")
            nc.sync.dma_start(out=m_sb, in_=maskR[:, :])

            # ---- weights/biases on the scalar queue (concurrent) ----
            ident = singles.tile([32, 32], BF16, tag="ident")
            make_identity(nc, ident)
            w1_sb = singles.tile([128, 12 * H1], BF16, tag="w1")
            nc.scalar.dma_start(out=w1_sb, in_=w1[:, :])
            w2_sb = singles.tile([128, 4 * H2], BF16, tag="w2")
            nc.scalar.dma_start(out=w2_sb, in_=w2[:, :])
            w3_sb = singles.tile([128, 2 * H3], BF16, tag="w3")
            nc.scalar.dma_start(out=w3_sb, in_=w3[:, :])
            bias_sb = singles.tile([128, 128], F32, tag="bias")
            nc.scalar.dma_start(out=bias_sb, in_=bias[:, :])
            b1_sb = bias_sb[:, 0:4]
            b2_sb = bias_sb[:, 4:6]
            b3_sb = bias_sb[:, 6:6 + H3]
            inva_sb = singles.tile([D, BL], F32, tag="inva")
            nc.scalar.dma_start(out=inva_sb, in_=inva[:, :])

            # persistent activations
            pooledT = work.tile([128, BL, 6, D], BF16, tag="pooledT")
            x1T = work.tile([128, 4, M], BF16, tag="x1T")
            x2T = work.tile([128, 2, M], BF16, tag="x2T")

            # ---- pooling + transpose per image ----
            for img in range(BL):
                ps_a = pps.tile([D, 384], F32, tag=f"pp{img}a")
                ps_b = pps.tile([D, 384], F32, tag=f"pp{img}b")
                for g in range(gs[img]):
                    f_sb = featp.tile([128, CG * C], BF16, tag="f")
                    nc.sync.dma_start(out=f_sb, in_=feats[img][g, :, :])
                    for gc in range(CG):
                        kk = g * CG + gc
                        if kk >= kchs[img]:
                            break
                        mk = m_sb[:, (moff[img] + kk) * D:(moff[img] + kk + 1) * D]
                        nc.tensor.matmul(ps_a, mk, f_sb[:, gc * C:gc * C + 384],
                                         start=(kk == 0),
                                         stop=(kk == kchs[img] - 1))
                        nc.tensor.matmul(ps_b, mk, f_sb[:, gc * C + 384:(gc + 1) * C],
                                         start=(kk == 0),
                                         stop=(kk == kchs[img] - 1))
                # scale by 1/area, cast to bf16
                pooled = tmpp.tile([D, C], BF16, tag="pooled")
                nc.vector.tensor_scalar_mul(pooled[:, 0:384], ps_a, inva_sb[:, img:img + 1])
                nc.vector.tensor_scalar_mul(pooled[:, 384:768], ps_b, inva_sb[:, img:img + 1])
                # transpose to [C, D] in 6 chunks of 128 channels
                for cc in range(6):
                    ps_t = mps.tile([128, D], BF16, tag="mm")
                    nc.tensor.transpose(ps_t, pooled[:, cc * 128:(cc + 1) * 128], ident)
                    nc.vector.tensor_copy(pooledT[:, img, cc, :], ps_t)

            # ---- layer 1 (factorized over pairs) ----
            for mc in range(4):
                ps_ab = mps.tile([128, BL, D], F32, tag="mm")
                for kc in range(6):
                    nc.tensor.matmul(ps_ab[:, :, 0:NH],
                                     w1_sb[:, kc * H1 + mc * 128:kc * H1 + (mc + 1) * 128],
                                     pooledT[:, :, kc, 0:NH],
                                     start=(kc == 0), stop=(kc == 5))
                for kc in range(6):
                    nc.tensor.matmul(ps_ab[:, :, NH:D],
                                     w1_sb[:, (6 + kc) * H1 + mc * 128:(6 + kc) * H1 + (mc + 1) * 128],
                                     pooledT[:, :, kc, NH:D],
                                     start=(kc == 0), stop=(kc == 5))
                ab_sb = tmpp.tile([128, BL, D], F32, tag="ab")
                nc.vector.tensor_copy(ab_sb, ps_ab)
                for img in range(BL):
                    pre = tmpp.tile([128, NH, NO], F32, tag="pre")
                    a_bc = ab_sb[:, img, 0:NH][:, :, None].broadcast_to([128, NH, NO])
                    b_bc = ab_sb[:, img, NH:D][:, None, :].broadcast_to([128, NH, NO])
                    # pre = (A + b1) + B
                    nc.vector.scalar_tensor_tensor(pre, a_bc, b1_sb[:, mc:mc + 1],
                                                   b_bc, op0=add, op1=add)
                    dst = x1T[:, mc, img * NPAIR:(img + 1) * NPAIR] \
                        .rearrange("p (i j) -> p i j", i=NH)
                    nc.vector.tensor_scalar_max(dst, pre, 0.0)

            # ---- layer 2 ----
            for m2 in range(2):
                ps2 = mps.tile([128, M], F32, tag="mm")
                for kc in range(4):
                    nc.tensor.matmul(ps2,
                                     w2_sb[:, kc * H2 + m2 * 128:kc * H2 + (m2 + 1) * 128],
                                     x1T[:, kc, :], start=(kc == 0), stop=(kc == 3))
                nc.vector.tensor_scalar(x2T[:, m2, :], ps2, b2_sb[:, m2:m2 + 1], 0.0,
                                        op0=add, op1=amax)

            # ---- layer 3 + bias + store ----
            for m3 in range(3):
                ps3 = mps.tile([128, H3], F32, tag="mm")
                for kc in range(2):
                    nc.tensor.matmul(ps3, x2T[:, kc, m3 * 128:(m3 + 1) * 128],
                                     w3_sb[:, kc * H3:(kc + 1) * H3],
                                     start=(kc == 0), stop=(kc == 1))
                o_sb = tmpp.tile([128, H3], F32, tag="osb")
                nc.vector.tensor_tensor(o_sb, ps3, b3_sb, op=add)
                nc.scalar.dma_start(out=out[m3 * 128:(m3 + 1) * 128, :], in_=o_sb)
    nc.compile()
    return nc


def _get_program(kchs):
    key = tuple(kchs)
    if key not in _PROGRAMS:
        _PROGRAMS[key] = _build_program(key)
    return _PROGRAMS[key]


def _preprocess(boxes, scores):
    """Box corners (reference's floor math), sorted detection order, 1/area,
    and per-image union-bbox crops."""
    cx, cy, bw, bh = boxes[..., 0], boxes[..., 1], boxes[..., 2], boxes[..., 3]
    x1 = np.floor((cx - bw / 2) * GRID).astype(np.int64)
    y1 = np.floor((cy - bh / 2) * GRID).astype(np.int64)
    x2 = np.floor((cx + bw / 2) * GRID).astype(np.int64)
    y2 = np.floor((cy + bh / 2) * GRID).astype(np.int64)
    hidx = np.argsort(-scores[:, :NH], axis=1, kind="stable")
    oidx = np.argsort(-scores[:, NH:], axis=1, kind="stable") + NH
    perm = np.concatenate([hidx, oidx], axis=1)                     # [B, D]
    g = np.arange(GRID)
    rows = (g[None, None, :] >= y1[..., None]) & (g[None, None, :] < y2[..., None])
    cols = (g[None, None, :] >= x1[..., None]) & (g[None, None, :] < x2[..., None])
    rows = np.take_along_axis(rows, perm[..., None], axis=1)        # [B, D, 64]
    cols = np.take_along_axis(cols, perm[..., None], axis=1)
    area = rows.sum(-1) * cols.sum(-1)                              # [B, D]
    # union-bbox crop windows (all mask support lies inside)
    r0 = np.clip(y1.min(axis=1), 0, GRID)
    r1 = np.clip(y2.max(axis=1), 0, GRID)
    c0 = np.clip(x1.min(axis=1), 0, GRID)
    c1 = np.clip(x2.max(axis=1), 0, GRID)
    crops = np.stack([r0, r1, c0, c1], axis=1)                      # [B, 4]
    return rows, cols, crops, (1.0 / area).astype(np.float32)


def _pack_image(feat_img, rows_img, cols_img, crop, kch):
    """Crop one image, flatten pixels, pad to kch*128, partition-major pack.
    Returns featR [G, 128, CG*C] bf16 and maskR [128, kchp*D] bf16."""
    r0, r1, c0, c1 = crop
    gpad = -(-kch // CG)
    kchp = gpad * CG
    npix = kchp * 128
    fc = feat_img[r0:r1, c0:c1, :].reshape(-1, C)
    p = fc.shape[0]
    fpad = np.zeros((npix, C), np.float32)
    fpad[:p] = fc
    featR = np.ascontiguousarray(
        fpad.reshape(gpad, CG, 128, C).transpose(0, 2, 1, 3)
        .reshape(gpad, 128, CG * C)).astype(BF)
    mask = (rows_img[:, r0:r1, None] & cols_img[:, None, c0:c1]) \
        .reshape(D, -1)                                             # [D, p]
    mpad = np.zeros((npix, D), np.float32)
    mpad[:p] = mask.T
    maskR = np.ascontiguousarray(
        mpad.reshape(kchp, 128, D).transpose(1, 0, 2).reshape(128, kchp * D)
    ).astype(BF)
    return featR, maskR


def _run(in_maps, trace=False, **kw):
    nc = _get_program(_LAST_META["kchs"])
    return run_bass_kernel_spmd(nc, in_maps, core_ids=list(range(N_CORES)),
                                trace=trace, **kw)


_LAST_META = {}


def _make_in_maps(features, boxes, scores, w1, b1, w2, b2, w3, b3):
    features = np.asarray(features, np.float32).reshape(B, GRID, GRID, C)
    rows, cols, crops, inva = _preprocess(np.asarray(boxes, np.float32),
                                          np.asarray(scores, np.float32))
    pix = (crops[:, 1] - crops[:, 0]) * (crops[:, 3] - crops[:, 2])
    kch_img = -(-pix // 128)                                        # [B]
    order = np.argsort(-kch_img, kind="stable")                     # big first
    # core c gets images (order[c], order[B-1-c]); slot KCH = slot max
    kchs = (int(kch_img[order[0]]), int(kch_img[order[N_CORES]]))
    _LAST_META["kchs"] = kchs
    _LAST_META["order"] = order

    # weights packed partition-major
    w1R = np.ascontiguousarray(
        np.asarray(w1, np.float32).reshape(12, 128, H1)
        .transpose(1, 0, 2).reshape(128, 12 * H1)).astype(BF)
    w2R = np.ascontiguousarray(
        np.asarray(w2, np.float32).reshape(4, 128, H2)
        .transpose(1, 0, 2).reshape(128, 4 * H2)).astype(BF)
    w3R = np.ascontiguousarray(
        np.asarray(w3, np.float32).reshape(2, 128, H3)
        .transpose(1, 0, 2).reshape(128, 2 * H3)).astype(BF)
    biasR = np.zeros((128, 128), np.float32)
    biasR[:, 0:4] = np.asarray(b1, np.float32).reshape(4, 128).T
    biasR[:, 4:6] = np.asarray(b2, np.float32).reshape(2, 128).T
    biasR[:, 6:6 + H3] = np.asarray(b3, np.float32)[None, :]

    in_maps = []
    for c in range(N_CORES):
        imgs = [order[c], order[B - 1 - c]]
        im = {"w1": w1R, "w2": w2R, "w3": w3R, "bias": biasR}
        mparts = []
        for j, b_idx in enumerate(imgs):
            featR, maskR = _pack_image(features[b_idx], rows[b_idx],
                                       cols[b_idx], crops[b_idx], kchs[j])
            im[f"feat{j}"] = featR
            mparts.append(maskR)
        im["maskR"] = np.ascontiguousarray(np.concatenate(mparts, axis=1))
        im["inva"] = np.ascontiguousarray(inva[imgs].T)             # [D, BL]
        in_maps.append(im)
    return in_maps


def kernel(features, boxes, scores, w1, b1, w2, b2, w3, b3, labels):
    in_maps = _make_in_maps(features, boxes, scores, w1, b1, w2, b2, w3, b3)
    res = _run(in_maps, trace=False)
    order = _LAST_META["order"]
    out = np.empty((B, NPAIR, H3), np.float32)
    for c in range(N_CORES):
        r = res.results[c]["out"].reshape(BL, NPAIR, H3)
        out[order[c]] = r[0]
        out[order[B - 1 - c]] = r[1]
    return np.ascontiguousarray(out)


# revision 10
# speedup vs baseline: 1.6764x; 1.0383x over previous
"""Trainium2 Bass kernel for the HOI relation model.

8 cores data-parallel over batch (2 images/core). Per core:
  1. ROI mean pooling over a host-packed stream of UNION pixels (only
     pixels covered by >=1 box, both images concatenated): KCH K-chunk
     matmuls with a combined 64-det block mask (cols 0-31 image A,
     32-63 image B), bf16 operands, f32 PSUM [64, 384] x2.
  2. PE-transpose pooled [64,768] -> pooledT [768,64] in 6 chunks; the
     transpose's "identity" operand is diag(1/area) so the mean scaling
     is free.
  3. Layer 1 factorized: relu(pair(h,o) @ w1 + b1) = relu(A(h) + B(o) + b1),
     pair expansion AFTER the matmul: DVE/GpSimd do the broadcast add,
     the scalar engine does relu(x + b1).
  4. Layers 2 (scalar-engine relu) and 3, single packed output store
     [128, 3*117] that the host unpacks.

DMA layout: everything is pre-packed host-side partition-major so each
dma_start is 128 descriptors of multi-KB contiguous runs. Features
stream on the sync-engine HWDGE queue; mask/diag + weights + biases on
the scalar-engine queue. The program is specialized on KCH (cached);
images are paired to cores so per-core pixel totals balance.

Host does only O(B*D + union-gather) prep: rasterization, argsort
order, 1/area, dtype casts, layout packing, shard/gather.
"""

import numpy as np
import ml_dtypes

import concourse.bass as bass
import concourse.mybir as mybir
import concourse.tile as tile
from concourse import bacc
from concourse.bass_utils import run_bass_kernel_spmd

N_CORES = 8
B, D, C = 16, 32, 768
NH, NO = 8, 24
NPAIR = NH * NO              # 192 pairs per image
GRID = 64                    # feature grid (896 / 14)
BL = 2                       # images per core
DD = BL * D                  # 64 combined detection columns
CG = 2                       # K-chunks per DMA tile
H1, H2, H3 = 512, 256, 117
M = BL * NPAIR               # 384 pair rows per core

F32 = mybir.dt.float32
BF16 = mybir.dt.bfloat16
BF = ml_dtypes.bfloat16
RELU = mybir.ActivationFunctionType.Relu

_PROGRAMS = {}               # KCH -> compiled Bacc


def _build_program(kch):
    g_n = -(-kch // CG)                       # feature DMA groups
    kchp = g_n * CG                           # padded chunk count
    add = mybir.AluOpType.add

    nc = bacc.Bacc("TRN2", target_bir_lowering=False, debug=False,
                   num_devices=N_CORES)
    feat = nc.declare_dram_parameter("feat", [g_n, 128, CG * C], BF16,
                                     isOutput=False)
    # mask blocks [128, kchp*DD] + one extra DD-wide block carrying
    # diag(1/area) in rows 0:64
    maskR = nc.declare_dram_parameter("maskR", [128, (kchp + 1) * DD], BF16,
                                      isOutput=False)
    w1 = nc.declare_dram_parameter("w1", [128, 12 * H1], BF16, isOutput=False)
    w2 = nc.declare_dram_parameter("w2", [128, 4 * H2], BF16, isOutput=False)
    w3 = nc.declare_dram_parameter("w3", [128, 2 * H3], BF16, isOutput=False)
    bias = nc.declare_dram_parameter("bias", [128, 128], F32, isOutput=False)
    out = nc.declare_dram_parameter("out", [128, 3 * H3], F32, isOutput=True)

    with tile.TileContext(nc) as tc:
        with (
            tc.tile_pool(name="singles", bufs=1) as singles,
            tc.tile_pool(name="featp", bufs=8) as featp,
            tc.tile_pool(name="work", bufs=1) as work,
            tc.tile_pool(name="tmp", bufs=3) as tmpp,
            tc.tile_pool(name="pps", bufs=1, space="PSUM") as pps,
            tc.tile_pool(name="mps", bufs=4, space="PSUM") as mps,
        ):
            # ---- scalar-queue loads: mask+diag first, then weights ----
            m_sb = singles.tile([128, (kchp + 1) * DD], BF16, tag="mask")
            nc.scalar.dma_start(out=m_sb, in_=maskR[:, :])
            diag_sb = m_sb[0:DD, kchp * DD:(kchp + 1) * DD]
            w1_sb = singles.tile([128, 12 * H1], BF16, tag="w1")
            nc.scalar.dma_start(out=w1_sb, in_=w1[:, :])
            w2_sb = singles.tile([128, 4 * H2], BF16, tag="w2")
            nc.scalar.dma_start(out=w2_sb, in_=w2[:, :])
            w3_sb = singles.tile([128, 2 * H3], BF16, tag="w3")
            nc.scalar.dma_start(out=w3_sb, in_=w3[:, :])
            bias_sb = singles.tile([128, 128], F32, tag="bias")
            nc.scalar.dma_start(out=bias_sb, in_=bias[:, :])
            b1_sb = bias_sb[:, 0:4]
            b2_sb = bias_sb[:, 4:6]
            b3_sb = bias_sb[:, 6:6 + H3]

            # persistent activations
            pooledT = work.tile([128, 6, BL, D], BF16, tag="pooledT")
            x1T = work.tile([128, 4, M], BF16, tag="x1T")
            x2T = work.tile([128, 2, M], BF16, tag="x2T")

            # ---- pooling: one combined accumulation over all chunks ----
            ps_a = pps.tile([DD, 384], F32, tag="ppa")
            ps_b = pps.tile([DD, 384], F32, tag="ppb")
            for g in range(g_n):
                f_sb = featp.tile([128, CG * C], BF16, tag="f")
                nc.sync.dma_start(out=f_sb, in_=feat[g, :, :])
                for gc in range(CG):
                    kk = g * CG + gc
                    if kk >= kch:
                        break
                    mk = m_sb[:, kk * DD:(kk + 1) * DD]
                    nc.tensor.matmul(ps_a, mk, f_sb[:, gc * C:gc * C + 384],
                                     start=(kk == 0), stop=(kk == kch - 1))
                    nc.tensor.matmul(ps_b, mk, f_sb[:, gc * C + 384:(gc + 1) * C],
                                     start=(kk == 0), stop=(kk == kch - 1))
            # psum -> sbuf bf16 (raw sums; scaling happens in transpose)
            pooled = tmpp.tile([DD, C], BF16, tag="pooled")
            nc.vector.tensor_copy(pooled[:, 0:384], ps_a)
            nc.scalar.activation(pooled[:, 384:768], ps_b,
                                 mybir.ActivationFunctionType.Copy)
            # transpose to [C, DD]; diag(1/area) applies the mean scale
            for cc in range(6):
                ps_t = mps.tile([128, DD], BF16, tag="mm")
                nc.tensor.transpose(ps_t, pooled[:, cc * 128:(cc + 1) * 128],
                                    diag_sb)
                dst = pooledT[:, cc, :, :]
                src = ps_t.rearrange("p (i d) -> p i d", i=BL)
                if cc % 2 == 0:
                    nc.vector.tensor_copy(dst, src)
                else:
                    nc.scalar.activation(dst, src,
                                         mybir.ActivationFunctionType.Copy)

            # ---- layer 1 (factorized over pairs, both images) ----
            for mc in range(4):
                ps_ab = mps.tile([128, BL, D], F32, tag="mm")
                for kc in range(6):
                    nc.tensor.matmul(ps_ab[:, :, 0:NH],
                                     w1_sb[:, kc * H1 + mc * 128:kc * H1 + (mc + 1) * 128],
                                     pooledT[:, kc, :, 0:NH],
                                     start=(kc == 0), stop=(kc == 5))
                for kc in range(6):
                    nc.tensor.matmul(ps_ab[:, :, NH:D],
                                     w1_sb[:, (6 + kc) * H1 + mc * 128:(6 + kc) * H1 + (mc + 1) * 128],
                                     pooledT[:, kc, :, NH:D],
                                     start=(kc == 0), stop=(kc == 5))
                ab_sb = tmpp.tile([128, BL, D], F32, tag="ab")
                if mc % 2 == 0:
                    nc.vector.tensor_copy(ab_sb, ps_ab)
                else:
                    nc.scalar.activation(ab_sb, ps_ab,
                                         mybir.ActivationFunctionType.Copy)
                for img in range(BL):
                    pre = tmpp.tile([128, NH, NO], F32, tag="pre")
                    a_bc = ab_sb[:, img, 0:NH][:, :, None].broadcast_to([128, NH, NO])
                    b_bc = ab_sb[:, img, NH:D][:, None, :].broadcast_to([128, NH, NO])
                    eng2 = nc.vector if img == 0 else nc.gpsimd
                    eng2.tensor_tensor(pre, a_bc, b_bc, op=add)
                    dst = x1T[:, mc, img * NPAIR:(img + 1) * NPAIR] \
                        .rearrange("p (i j) -> p i j", i=NH)
                    nc.scalar.activation(dst, pre, RELU, bias=b1_sb[:, mc:mc + 1])

            # ---- layer 2 ----
            for m2 in range(2):
                ps2 = mps.tile([128, M], F32, tag="mm")
                for kc in range(4):
                    nc.tensor.matmul(ps2,
                                     w2_sb[:, kc * H2 + m2 * 128:kc * H2 + (m2 + 1) * 128],
                                     x1T[:, kc, :], start=(kc == 0), stop=(kc == 3))
                nc.scalar.activation(x2T[:, m2, :], ps2, RELU,
                                     bias=b2_sb[:, m2:m2 + 1])

            # ---- layer 3 + bias + single store ----
            ps3 = pps.tile([128, 3, H3], F32, tag="ps3")
            for m3 in range(3):
                for kc in range(2):
                    nc.tensor.matmul(ps3[:, m3, :], x2T[:, kc, m3 * 128:(m3 + 1) * 128],
                                     w3_sb[:, kc * H3:(kc + 1) * H3],
                                     start=(kc == 0), stop=(kc == 1))
            o_sb = tmpp.tile([128, 3, H3], F32, tag="osb")
            b3_bc = b3_sb[:, None, :].broadcast_to([128, 3, H3])
            nc.vector.tensor_tensor(o_sb, ps3, b3_bc, op=add)
            nc.scalar.dma_start(out=out[:, :],
                                in_=o_sb.rearrange("p a b -> p (a b)"))
    nc.compile()
    return nc


def _get_program(kch):
    if kch not in _PROGRAMS:
        _PROGRAMS[kch] = _build_program(kch)
    return _PROGRAMS[kch]


def _preprocess(boxes, scores):
    """Box corners (reference's floor math), sorted det order, 1/area,
    and per-image union pixel coordinates."""
    cx, cy, bw, bh = boxes[..., 0], boxes[..., 1], boxes[..., 2], boxes[..., 3]
    x1 = np.floor((cx - bw / 2) * GRID).astype(np.int64)
    y1 = np.floor((cy - bh / 2) * GRID).astype(np.int64)
    x2 = np.floor((cx + bw / 2) * GRID).astype(np.int64)
    y2 = np.floor((cy + bh / 2) * GRID).astype(np.int64)
    hidx = np.argsort(-scores[:, :NH], axis=1, kind="stable")
    oidx = np.argsort(-scores[:, NH:], axis=1, kind="stable") + NH
    perm = np.concatenate([hidx, oidx], axis=1)                     # [B, D]
    g = np.arange(GRID)
    rows = (g[None, None, :] >= y1[..., None]) & (g[None, None, :] < y2[..., None])
    cols = (g[None, None, :] >= x1[..., None]) & (g[None, None, :] < x2[..., None])
    rows = np.take_along_axis(rows, perm[..., None], axis=1)        # [B, D, 64]
    cols = np.take_along_axis(cols, perm[..., None], axis=1)
    area = rows.sum(-1) * cols.sum(-1)                              # [B, D]
    cover = np.einsum('bdy,bdx->byx', rows, cols) > 0               # [B, 64, 64]
    return rows, cols, cover, (1.0 / area).astype(np.float32)


_LAST_META = {}


def _make_in_maps(features, boxes, scores, w1, b1, w2, b2, w3, b3):
    features = np.asarray(features, np.float32).reshape(B, GRID, GRID, C)
    rows, cols, cover, inva = _preprocess(np.asarray(boxes, np.float32),
                                          np.asarray(scores, np.float32))
    # union pixel coords per image (row-major order)
    pys = [np.nonzero(cover[b]) for b in range(B)]
    pcount = np.array([len(p[0]) for p in pys])
    # pair images to balance per-core totals: sort desc, pair i with B-1-i
    order = np.argsort(-pcount, kind="stable")
    pairs = [(order[c], order[B - 1 - c]) for c in range(N_CORES)]
    kch = int(max(-(-(pcount[a] + pcount[b]) // 128) for a, b in pairs))
    g_n = -(-kch // CG)
    kchp = g_n * CG
    _LAST_META["kch"] = kch
    _LAST_META["pairs"] = pairs

    w1R = np.ascontiguousarray(
        np.asarray(w1, np.float32).reshape(12, 128, H1)
        .transpose(1, 0, 2).reshape(128, 12 * H1)).astype(BF)
    w2R = np.ascontiguousarray(
        np.asarray(w2, np.float32).reshape(4, 128, H2)
        .transpose(1, 0, 2).reshape(128, 4 * H2)).astype(BF)
    w3R = np.ascontiguousarray(
        np.asarray(w3, np.float32).reshape(2, 128, H3)
        .transpose(1, 0, 2).reshape(128, 2 * H3)).astype(BF)
    biasR = np.zeros((128, 128), np.float32)
    biasR[:, 0:4] = np.asarray(b1, np.float32).reshape(4, 128).T
    biasR[:, 4:6] = np.asarray(b2, np.float32).reshape(2, 128).T
    biasR[:, 6:6 + H3] = np.asarray(b3, np.float32)[None, :]

    in_maps = []
    npix = kchp * 128
    for a, bidx in pairs:
        fpad = np.zeros((npix, C), np.float32)
        mpad = np.zeros((npix, DD), np.float32)
        off = 0
        for j, bi in enumerate((a, bidx)):
            yy, xx = pys[bi]
            p = len(yy)
            fpad[off:off + p] = features[bi][yy, xx]
            # mask[pix, d] = (rows[d, y] & cols[d, x]) / area_d, det cols
            # at j*D -- 1/area folded in so pooling psum = mean directly
            mpad[off:off + p, j * D:(j + 1) * D] = \
                (rows[bi][:, yy] & cols[bi][:, xx]).T * inva[bi][None, :]
            off += p
        featR = np.ascontiguousarray(
            fpad.reshape(g_n, CG, 128, C).transpose(0, 2, 1, 3)
            .reshape(g_n, 128, CG * C)).astype(BF)
        maskR = np.zeros((128, (kchp + 1) * DD), np.float32)
        maskR[:, :kchp * DD] = \
            mpad.reshape(kchp, 128, DD).transpose(1, 0, 2).reshape(128, -1)
        dg = np.zeros((128, DD), np.float32)
        dg[np.arange(DD), np.arange(DD)] = 1.0
        maskR[:, kchp * DD:] = dg
        in_maps.append({
            "feat": featR,
            "maskR": np.ascontiguousarray(maskR).astype(BF),
            "w1": w1R, "w2": w2R, "w3": w3R, "bias": biasR,
        })
    return in_maps


def _run(in_maps, trace=False, **kw):
    nc = _get_program(_LAST_META["kch"])
    return run_bass_kernel_spmd(nc, in_maps, core_ids=list(range(N_CORES)),
                                trace=trace, **kw)


def kernel(features, boxes, scores, w1, b1, w2, b2, w3, b3, labels):
    in_maps = _make_in_maps(features, boxes, scores, w1, b1, w2, b2, w3, b3)
    res = _run(in_maps, trace=False)
    out = np.empty((B, NPAIR, H3), np.float32)
    for c, (a, bidx) in enumerate(_LAST_META["pairs"]):
        r = res.results[c]["out"].reshape(128, 3, H3) \
            .transpose(1, 0, 2).reshape(M, H3)
        out[a] = r[0:NPAIR]
        out[bidx] = r[NPAIR:M]
    return np.ascontiguousarray(out)
